# revision 4
# baseline (speedup 1.0000x reference)
"""Trainium2 Bass kernel for nn_LIF_hh_neuron (B=2048, T=15, IN=512, C=1024).

Sharding: pure data-parallel over batch B across 8 NeuronCores (256 each).

Per core:
  - p_k(t) = x_t @ W_k.T runs on PE as a 3-pass hi/lo split for fp32-class
    precision at full PE rate: xh(f32r)@Wh(f32r) + xh(bf16)@Wl(bf16) +
    xl(f32r)@Wh(f32r), where h/l split W and x on the f32r (11-bit
    mantissa) grid.  Measured max abs err ~5e-7 vs 2e-4 for 1-pass f32r.
  - LIF state update per timestep stays on-chip:
      mem(t)   = psum(t) + upb(t)                  [DVE tensor_tensor]
      spike(t) = mem(t) > 0.8 -> interleaved (c,j) [DVE tensor_scalar]
      upb(t+1) = select(mem<=0.8, 0.2*mem, 0) + b  [custom DVE op]
      v_w      = wl_w*mem_w (+bl for w=0)          [ACT scaled copies]
      z(t+1)   = v0+v1+v2                          [GPSIMD adds]
      plane 3 (no matmul): s3=(z+u3)>0.8, u3'=gate [2-src custom DVE ops]
"""

import numpy as np
import ml_dtypes

import concourse.bass as bass
import concourse.mybir as mybir
import concourse.tile as tile
from concourse import bacc
from concourse import bass_utils
from concourse.masks import make_identity
from concourse.dve_spec import Spec, Src0, Src1, C0, C1, Zero, select, lower
from concourse.dve_ops import has_src1, DveOp, OPS
import concourse.dve_ops as dve_ops_mod
from concourse.dve_uop import DveOpSpec

F32 = mybir.dt.float32
F32R = mybir.dt.float32r
BF16 = mybir.dt.bfloat16

B, T, IN, C = 2048, 15, 512, 1024
NCORES = 8
BLOC = B // NCORES          # 256 batches per core
NBT = BLOC // 128           # 2 batch tiles per core
KC = IN // 128              # 4 contraction chunks
NH = 2                      # two c-halves of 512
THRESH = 0.8
DECAY = 0.2


def _register_op(name, spec, subdim=False):
    for existing in OPS:
        if existing.name == name:
            return existing
    op = DveOp(name, spec, subdim=subdim, uops_sha={})
    OPS.append(op)
    dve_ops_mod._SUB_OPCODE_FOR_NAME[name] = (
        dve_ops_mod._CUSTOM_DVE_ROW_BASE + len(OPS) - 1
    )
    dve_ops_mod.CUSTOM_DVE_SPECS[name] = spec
    shas = {}
    for ver in ("v3", "v4"):
        s = DveOpSpec(
            name=name,
            opcode=dve_ops_mod.get_dve_sub_opcode(name),
            uops=lower(spec, ver=ver),
            rd1_en=has_src1(spec),
        )
        shas[ver] = s.sha(ver)
    object.__setattr__(op, "uops_sha", shas)
    return op


# upb = select(mem <= thr, mem*decay, 0) + b
LIF_UPB = _register_op(
    "LIF_UPB_ANT",
    Spec(
        body=select(Src0 <= C0, Src0 * C1, Zero) + Src1,
        reference=lambda in0, in1, s0, s1: (
            np.where(in0 <= s0, in0 * s1, 0.0) + in1
        ).astype(np.float32),
    ),
)

# s3 = (z + u3) > thr
LIF_SPIKE2 = _register_op(
    "LIF_SPIKE2_ANT",
    Spec(
        body=(Src0 + Src1) > C0,
        reference=lambda in0, in1, s0, s1: ((in0 + in1) > s0).astype(np.float32),
    ),
)

# u3' = select(z + u3 <= thr, (z + u3)*decay, 0)
LIF_GATE2 = _register_op(
    "LIF_GATE2_ANT",
    Spec(
        body=select((Src0 + Src1) <= C0, (Src0 + Src1) * C1, Zero),
        reference=lambda in0, in1, s0, s1: np.where(
            (in0 + in1) <= s0, (in0 + in1) * s1, 0.0
        ).astype(np.float32),
    ),
)


def _round11(a):
    """Round fp32 mantissa to 11 explicit bits (the f32r grid), nearest-even."""
    u = np.ascontiguousarray(a, np.float32).view(np.uint32)
    half = np.uint32(1 << 11)
    mask = np.uint32((1 << 12) - 1)
    frac = u & mask
    u2 = u & ~mask
    rup = (frac > half) | (
        (frac == half) & ((u2 >> np.uint32(12)) & np.uint32(1)).astype(bool)
    )
    return (u2 + np.where(rup, np.uint32(1 << 12), np.uint32(0))).view(np.float32)


def _build(passes=3):
    nc = bacc.Bacc("TRN2", target_bir_lowering=False, debug=False)

    d_x = nc.dram_tensor("x", [BLOC, T, IN], F32, kind="ExternalInput").ap()
    d_wh = nc.dram_tensor("wh", [KC, 128, 3, C], F32, kind="ExternalInput").ap()
    d_wl16 = nc.dram_tensor("wl16", [KC, 128, 3, C], BF16, kind="ExternalInput").ap()
    d_b = nc.dram_tensor("b", [1, 3, C], F32, kind="ExternalInput").ap()
    d_wlb = nc.dram_tensor("wlb", [1, 4], F32, kind="ExternalInput").ap()
    d_out = nc.dram_tensor("spk", [BLOC, T, 4 * C], F32, kind="ExternalOutput").ap()

    nchain = NBT * NH

    with tile.TileContext(nc) as tc:
        with (
            tc.tile_pool(name="wpool", bufs=1) as wpool,
            tc.tile_pool(name="state", bufs=1) as state,
            tc.tile_pool(name="mem", bufs=1) as mempool,
            tc.tile_pool(name="upb", bufs=1) as upbpool,
            tc.tile_pool(name="vpool", bufs=2) as vpool,
            tc.tile_pool(name="spool", bufs=2) as spool,
            tc.tile_pool(name="xin", bufs=2) as xin,
            tc.tile_pool(name="xtp", bufs=2) as xtp,
            tc.tile_pool(name="pspool", bufs=2, space="PSUM") as pspool,
            tc.tile_pool(name="trps", bufs=2, space="PSUM") as trps,
        ):
            # ---- static tiles ----
            t_wh = wpool.tile([128, KC, 3, C], F32R, tag="wh")
            nc.sync.dma_start(
                out=t_wh, in_=d_wh.rearrange("k p w c -> p k w c").bitcast(F32R)
            )
            t_wl16 = wpool.tile([128, KC, 3, C], BF16, tag="wl16")
            nc.sync.dma_start(out=t_wl16, in_=d_wl16.rearrange("k p w c -> p k w c"))
            t_b = wpool.tile([128, 3, C], F32, tag="b")
            nc.sync.dma_start(
                out=t_b,
                in_=bass.AP(tensor=d_b.tensor, offset=0, ap=[[0, 128], [C, 3], [1, C]]),
            )
            t_wlb = wpool.tile([128, 4], F32, tag="wlb")
            nc.sync.dma_start(
                out=t_wlb,
                in_=bass.AP(tensor=d_wlb.tensor, offset=0, ap=[[0, 128], [1, 4]]),
            )
            t_id = wpool.tile([128, 128], F32, tag="ident")
            make_identity(nc, t_id)

            # ---- per-chain recurrent state ----
            t_z = [
                state.tile([128, 512], F32, tag=f"z{ch}", name=f"z{ch}")
                for ch in range(nchain)
            ]
            t_u3 = [
                state.tile([128, 512], F32, tag=f"u3{ch}", name=f"u3{ch}")
                for ch in range(nchain)
            ]
            for ch in range(nchain):
                nc.vector.memset(t_u3[ch], 0.0)
                # z(0) = bl  (Copy(1*0 + bl) reading the zeroed u3)
                nc.scalar.activation(
                    t_z[ch],
                    t_u3[ch],
                    mybir.ActivationFunctionType.Identity,
                    bias=t_wlb[:, 3:4],
                    scale=1.0,
                )

            upb_prev = [None] * nchain  # None -> b tile itself at t=0

            for t in range(T):
                for bt in range(NBT):
                    b0 = bt * 128
                    x_t = xin.tile([128, IN], F32, tag="x")
                    nc.sync.dma_start(out=x_t, in_=d_x[b0 : b0 + 128, t, :])

                    # PE transpose of raw x_t -> PSUM [128(k), KC, 128(b)]
                    trx = trps.tile([128, KC, 128], F32, tag="trx")
                    for k in range(KC):
                        nc.tensor.transpose(
                            trx[:, k, :], x_t[:, k * 128 : (k + 1) * 128], t_id
                        )
                    # hi/lo split on the transposed copy
                    xhT = xtp.tile([128, KC, 128], F32R, tag="xhT")
                    nc.vector.tensor_copy(xhT, trx)  # rounds to f32r grid
                    xlT = xtp.tile([128, KC, 128], F32R, tag="xlT")
                    nc.vector.tensor_tensor(
                        out=xlT, in0=trx, in1=xhT.bitcast(F32),
                        op=mybir.AluOpType.subtract,
                    )
                    xhT16 = xtp.tile([128, KC, 128], BF16, tag="xhT16")
                    nc.vector.tensor_copy(xhT16, xhT.bitcast(F32))

                    for h in range(NH):
                        ch = bt * NH + h
                        c0 = h * 512
                        ps = pspool.tile([128, 3, 512], F32, tag="ps")
                        for w in range(3):
                            for k in range(KC):
                                nc.tensor.matmul(
                                    ps[:, w, :],
                                    xhT[:, k, :],
                                    t_wh[:, k, w, c0 : c0 + 512],
                                    start=(k == 0),
                                    stop=False,
                                )
                            if passes >= 2:
                                for k in range(KC):
                                    nc.tensor.matmul(
                                        ps[:, w, :],
                                        xhT16[:, k, :],
                                        t_wl16[:, k, w, c0 : c0 + 512],
                                        start=False,
                                        stop=(passes == 2 and k == KC - 1),
                                    )
                            if passes >= 3:
                                for k in range(KC):
                                    nc.tensor.matmul(
                                        ps[:, w, :],
                                        xlT[:, k, :],
                                        t_wh[:, k, w, c0 : c0 + 512],
                                        start=False,
                                        stop=(k == KC - 1),
                                    )

                        # mem(t) = psum + upb(t)
                        mem_t = mempool.tile([128, 3, 512], F32, tag=f"mem{ch}")
                        ub = upb_prev[ch]
                        if ub is None:
                            ub = t_b[:, :, c0 : c0 + 512]
                        nc.vector.tensor_tensor(
                            out=mem_t, in0=ps, in1=ub, op=mybir.AluOpType.add
                        )

                        # spikes, interleaved (c, j) in one tile [128, 512, 4]
                        S = spool.tile([128, 512, 4], F32, tag="S")
                        S_jc = S.rearrange("p c j -> p j c")
                        nc.vector.tensor_scalar(
                            out=S_jc[:, 0:3, :],
                            in0=mem_t,
                            scalar1=THRESH,
                            scalar2=None,
                            op0=mybir.AluOpType.is_gt,
                        )
                        z3 = t_z[ch].rearrange("p (s n) -> p s n", s=1)
                        u33 = t_u3[ch].rearrange("p (s n) -> p s n", s=1)
                        nc.vector._custom_dve(
                            LIF_SPIKE2, out=S_jc[:, 3:4, :], in0=z3, in1=u33, s0=THRESH
                        )
                        nc.vector._custom_dve(
                            LIF_GATE2, out=u33, in0=z3, in1=u33, s0=THRESH, s1=DECAY
                        )

                        if t < T - 1:
                            upb_t = upbpool.tile([128, 3, 512], F32, tag=f"upb{ch}")
                            nc.vector._custom_dve(
                                LIF_UPB,
                                out=upb_t,
                                in0=mem_t,
                                in1=t_b[:, :, c0 : c0 + 512],
                                s0=THRESH,
                                s1=DECAY,
                            )
                            upb_prev[ch] = upb_t

                            v = vpool.tile([128, 3, 512], F32, tag="v")
                            for w in range(3):
                                nc.scalar.activation(
                                    v[:, w, :],
                                    mem_t[:, w, :],
                                    mybir.ActivationFunctionType.Identity,
                                    bias=t_wlb[:, 3:4] if w == 0 else 0.0,
                                    scale=t_wlb[:, w : w + 1],
                                )
                            zt = vpool.tile([128, 512], F32, tag="ztmp")
                            nc.gpsimd.tensor_tensor(
                                out=zt,
                                in0=v[:, 0, :],
                                in1=v[:, 1, :],
                                op=mybir.AluOpType.add,
                            )
                            nc.gpsimd.tensor_tensor(
                                out=t_z[ch],
                                in0=zt,
                                in1=v[:, 2, :],
                                op=mybir.AluOpType.add,
                            )

                        nc.sync.dma_start(
                            out=d_out[b0 : b0 + 128, t, c0 * 4 : c0 * 4 + 2048],
                            in_=S.rearrange("p c j -> p (c j)"),
                        )

    nc.finalize()
    return nc


_NC_CACHE = {}


def _get_nc(passes=3):
    if passes not in _NC_CACHE:
        _NC_CACHE[passes] = _build(passes)
    return _NC_CACHE[passes]


def kernel(**inputs):
    x = np.ascontiguousarray(np.asarray(inputs["x"], dtype=np.float32))
    W = [np.asarray(inputs[f"W{i}"], dtype=np.float32) for i in (1, 2, 3)]
    bvec = [np.asarray(inputs[f"b{i}"], dtype=np.float32) for i in (1, 2, 3)]
    Wl = np.asarray(inputs["Wl"], dtype=np.float32)
    bl = np.asarray(inputs["bl"], dtype=np.float32)

    WT = np.stack([Wk.T for Wk in W], axis=1).astype(np.float32)  # [IN, 3, C]
    Wh = _round11(WT)
    Wl16 = (WT - Wh).astype(ml_dtypes.bfloat16)
    wh = np.ascontiguousarray(Wh.reshape(KC, 128, 3, C))
    wl16 = np.ascontiguousarray(Wl16.reshape(KC, 128, 3, C))
    b_cat = np.ascontiguousarray(np.stack(bvec, axis=0).reshape(1, 3, C))
    wlb = np.concatenate([Wl[0].reshape(3), bl.reshape(1)]).reshape(1, 4).astype(
        np.float32
    )

    nc = _get_nc(3)
    in_maps = [
        dict(x=x[c * BLOC : (c + 1) * BLOC], wh=wh, wl16=wl16, b=b_cat, wlb=wlb)
        for c in range(NCORES)
    ]
    res = bass_utils.run_bass_kernel_spmd(nc, in_maps, core_ids=list(range(NCORES)))
    return np.concatenate([r["spk"] for r in res.results], axis=0)


if __name__ == "__main__":
    rng = np.random.default_rng(0)
    s_in = 1.0 / np.sqrt(IN)
    s3 = 1.0 / np.sqrt(3.0)
    ins = dict(
        x=rng.standard_normal((B, T, IN)).astype(np.float32),
        W1=rng.uniform(-s_in, s_in, (C, IN)).astype(np.float32),
        b1=rng.uniform(-s_in, s_in, (C,)).astype(np.float32),
        W2=rng.uniform(-s_in, s_in, (C, IN)).astype(np.float32),
        b2=rng.uniform(-s_in, s_in, (C,)).astype(np.float32),
        W3=rng.uniform(-s_in, s_in, (C, IN)).astype(np.float32),
        b3=rng.uniform(-s_in, s_in, (C,)).astype(np.float32),
        Wl=rng.uniform(-s3, s3, (1, 3)).astype(np.float32),
        bl=rng.uniform(-s3, s3, (1,)).astype(np.float32),
        wins=T,
    )
    out = kernel(**ins)

    # numpy reference
    p = [
        (ins["x"].reshape(B * T, IN) @ ins[f"W{k+1}"].T + ins[f"b{k+1}"]).reshape(
            B, T, C
        )
        for k in range(3)
    ]
    mem = np.zeros((B, C, 4), np.float32)
    spk = np.zeros((B, C, 4), np.float32)
    exp = np.zeros((B, T, C, 4), np.float32)
    for t in range(T):
        inner = mem[..., :3] @ ins["Wl"][0] + ins["bl"][0]
        ia = np.stack([p[0][:, t], p[1][:, t], p[2][:, t], inner], axis=-1)
        mem = mem * np.float32(0.2) * (1.0 - spk) + ia
        spk = (mem > 0.8).astype(np.float32)
        exp[:, t] = spk
    exp = exp.reshape(B, T, C * 4)
    rel = np.linalg.norm(out - exp) / np.linalg.norm(exp)
    print("out", out.shape, out.dtype, "density", out.mean())
    print("rel err vs numpy fp32:", rel, "nflips", np.abs(out - exp).sum())
